# revision 31
# baseline (speedup 1.0000x reference)
"""Trainium2 Bass kernel for nn_Conv_48679159332865 (Chebyshev spectral graph conv).

Algorithm (per core, data-parallel over the B*X*Y*Z dense dim):
  out = sum_k Cheb_k(L) @ x0 @ W_k + bias
evaluated via Clenshaw's backward recurrence:
  B_k = U_k + 2 L B_{k+1} - B_{k+2}   for k = 6..1  (B_7 = U_7, B_8 = 0)
  S   = U_0 + L B_1 - B_2 + bias
with U_k = x0 @ W_k fused into each sweep as small PE matmuls.

The sparse laplacian (36864 nnz of 16.7M) is NOT densified. Each sweep,
every 128-row output tile gathers the ~1010 distinct source rows it needs
from the previous Clenshaw level (held in HBM, bf16, rows padded to 896
cols so each gather descriptor is 1792B) via the SWDGE dma_gather path,
then reduces them on the PE against static "sel" stationaries that carry
the laplacian values: psum[r,:] += sum_j sel_j[s,r] * G[s,j,:]. This cuts
PE work ~3.5x vs dense-L and makes the kernel HBM/DMA bound (the
target regime). Levels ping-pong between two HBM slabs; the -B_{k+2}
term streams back tile-aligned from the slab about to be overwritten.
Each of the 8 cores handles 27 of the 216 dense columns; the laplacian
schedule (indices + sel matrices) is replicated.
"""

import sys
from contextlib import ExitStack

import numpy as np
import ml_dtypes

for _p in ("/opt/trn_rl_repo", "/root/.axon_site/_ro/trn_rl_repo"):
    if _p not in sys.path:
        sys.path.insert(0, _p)

import concourse.bass as bass
import concourse.tile as tile
from concourse.tile import add_dep_helper
from concourse import mybir
from concourse import bass2jax
from concourse.library_config import mlp
from concourse.library_overlay import lower_extended_insts

FIN, V, FOUT, KK = 32, 4096, 32, 8
DP = 216            # B*X*Y*Z dense positions
NCORES = 8
DPC = DP // NCORES  # 27 dense positions per core
DC = DPC * FIN      # 864 working columns per core
DCP = 896           # padded row length for gather (896*2B = 7*256)
NT = V // 128       # 32 v-tiles
BF16 = mybir.dt.bfloat16
F32 = mybir.dt.float32
I16 = mybir.dt.int16

MUL = mybir.AluOpType.mult
ADD = mybir.AluOpType.add
SUB = mybir.AluOpType.subtract

_CACHE = {}
GATHER_MODE = "indirect"  # "indirect" (HW DGE) or "swdge" (Q7 desc-gen)


def _fix_excess_waits(nc, limit=1):
    """This walrus build supports one sync-wait per instruction; hoist excess
    waits onto NoOps inserted before the offending instruction."""
    for f in nc.m.functions:
        for blk in f.blocks:
            new_insts = []
            for inst in blk.instructions:
                si = inst.sync_info
                if si is not None and si.on_wait and len(si.on_wait) > limit:
                    waits = list(si.on_wait)
                    extra, keep = waits[:-limit], waits[-limit:]
                    for i in range(0, len(extra), limit):
                        nop = mybir.InstNoOp(
                            name=f"{inst.name}-waitsplit-{i}", ins=[], outs=[]
                        )
                        nop.engine = inst.engine
                        nop.sync_info = mybir.SyncInfo(
                            on_wait=extra[i : i + limit], on_update=[]
                        )
                        nc.register_instruction(nop, overwrite=True)
                        new_insts.append(nop)
                    inst.sync_info = mybir.SyncInfo(
                        on_wait=keep, on_update=list(si.on_update)
                    )
                new_insts.append(inst)
            blk.instructions[:] = new_insts


def _build_schedule(lap_rows, lap_cols, lap_vals):
    """Per-dst-tile gather/sel schedule from the COO laplacian.

    Returns (tiles, sel, gidx) where tiles[I] = dict(jt, n, scol, icol),
    sel is the [128, sum(jt)*128] bf16 stationary bank (partition = slot
    within group, col = group*128 + local dst row), gidx the wrapped int16
    index bank [128, sum(jt)*8].
    """
    bf = ml_dtypes.bfloat16
    r = np.asarray(lap_rows, dtype=np.int64)
    c = np.asarray(lap_cols, dtype=np.int64)
    v = np.asarray(lap_vals, dtype=np.float64)
    key = r * V + c
    uk, inv = np.unique(key, return_inverse=True)
    uval = np.bincount(inv, weights=v)
    ur = uk // V
    uc = uk % V

    tiles = []
    sel_blocks = []
    idx_lists = []
    for I in range(NT):
        m = (ur // 128) == I
        er = (ur[m] - 128 * I).astype(np.int64)
        ec = uc[m]
        ev = uval[m].astype(np.float32)
        srcs = np.unique(ec)  # sorted ascending: better HBM locality
        n = len(srcs)
        jt = max(1, -(-n // 128))
        slot = np.searchsorted(srcs, ec)
        blk = np.zeros((jt * 128, 128), np.float32)
        blk[slot, er] = ev
        # pad with dummy valid index 0 (sel rows stay zero) so every gather
        # moves exactly jt*128 rows — lets one shared register carry
        # num_idxs_reg for all tiles with the same jt.
        idx = np.zeros(jt * 128, np.int16)
        idx[:n] = srcs.astype(np.int16)
        tiles.append({"jt": jt, "n": int(n)})
        sel_blocks.append(blk)
        idx_lists.append(idx)

    scol = 0
    icol = 0
    gcol = 0
    for I, t in enumerate(tiles):
        t["scol"] = scol
        t["icol"] = icol
        t["gcol"] = gcol
        scol += t["jt"] * 128
        icol += t["jt"] * 8
        gcol += t["jt"]

    SELW = scol
    GIW = icol
    sel = np.zeros((128, SELW), np.float32)
    gidx = np.zeros((128, GIW), np.int16)
    gidx32 = np.zeros((128, gcol), np.int32)
    for I, t in enumerate(tiles):
        blk = sel_blocks[I]
        for j in range(t["jt"]):
            sel[:, t["scol"] + j * 128 : t["scol"] + (j + 1) * 128] = (
                blk[j * 128 : (j + 1) * 128, :]
            )
        idx = idx_lists[I]
        w = idx.reshape(-1, 16).T  # [16, jt*8]: slot i at (i%16, i//16)
        for g in range(8):
            gidx[g * 16 : (g + 1) * 16, t["icol"] : t["icol"] + t["jt"] * 8] = w
        # per-partition layout for HW-DGE indirect gathers: col = group,
        # partition p holds the src row for slot p of that group
        gidx32[:, t["gcol"] : t["gcol"] + t["jt"]] = (
            idx.reshape(t["jt"], 128).T.astype(np.int32)
        )
    return tiles, np.ascontiguousarray(sel.astype(bf)), gidx, gidx32


def _build_nc(tiles, npasses=1, skip=()):
    """npasses>1 runs the whole algorithm that many times back-to-back —
    used only for measuring device execution time by differencing.
    skip: diagnostic stage-skipping for perf attribution ("gather",
    "spmm", "umat", "combine", "io"); results are garbage when nonempty."""
    nc = bass.Bass(
        "TRN2", target_bir_lowering=False, debug=False, num_swdge_queues=4
    )
    SELW = sum(t["jt"] for t in tiles) * 128
    GIW = sum(t["jt"] for t in tiles) * 8
    JTMAX = max(t["jt"] for t in tiles)

    TOTG = sum(t["jt"] for t in tiles)
    x0t = nc.dram_tensor("x0t", [DC, V], BF16, kind="ExternalInput")
    wk = nc.dram_tensor("wk", [128, KK * 128], BF16, kind="ExternalInput")
    seld = nc.dram_tensor("seld", [128, SELW], BF16, kind="ExternalInput")
    gidxd = nc.dram_tensor("gidxd", [128, GIW], I16, kind="ExternalInput")
    gidx32d = nc.dram_tensor(
        "gidx32d", [128, TOTG], mybir.dt.int32, kind="ExternalInput"
    )
    brep = nc.dram_tensor("brep", [128, DC], F32, kind="ExternalInput")
    sout = nc.dram_tensor("sout", [V, DC], BF16, kind="ExternalOutput")
    bb = [
        nc.dram_tensor(f"bb{s}", [V, DCP], BF16, kind="Internal") for s in range(2)
    ]

    with tile.TileContext(nc, pool_alloc_mode="queue") as tc, ExitStack() as ctx:
      prev_tail = None  # fence on the previous pass's sout writes
      for _pass in range(npasses):
        cp = ctx.enter_context(tc.tile_pool(name=f"const{_pass}", bufs=1))
        nc.gpsimd.load_library(mlp)
        # one dma_gather moves at most 256 rows (payload must stay under
        # the 512KB per-instruction descriptor-ring budget at 1792B/row)
        nreg = {n: nc.gpsimd.to_reg(n * 128) for n in (1, 2)}
        x0sb = [
            cp.tile([128, V], BF16, name=f"x0_{r}", tag=f"x0_{r}")
            for r in range(7)
        ]
        nc.vector.memset(x0sb[6][96:128, :], 0.0)
        for r in range(7):
            rows = 128 if r < 6 else 96
            nc.scalar.dma_start(
                x0sb[r][0:rows, :], x0t.ap()[r * 128 : r * 128 + rows, :]
            )
        wksb = cp.tile([128, KK * 128], BF16, tag="wk")
        nc.scalar.dma_start(wksb[:], wk.ap())
        brsb = cp.tile([128, DC], F32, tag="br")
        nc.scalar.dma_start(brsb[:], brep.ap())
        gisb = cp.tile([128, GIW], I16, tag="gidx")
        nc.sync.dma_start(gisb[:], gidxd.ap())
        g32sb = cp.tile([128, TOTG], mybir.dt.int32, tag="gidx32")
        nc.sync.dma_start(g32sb[:], gidx32d.ap())
        selsb = cp.tile([128, SELW], BF16, tag="sel")
        half = (SELW // 2) // 128 * 128
        nc.sync.dma_start(selsb[:, :half], seld.ap()[:, :half])
        nc.sync.dma_start(selsb[:, half:], seld.ap()[:, half:])
        # zero the pad columns (864:896) of both HBM slabs once; kernel
        # writes only [:, :864] afterwards so they stay zero.
        zt = cp.tile([128, NT * (DCP - DC)], BF16, tag="zpad")
        nc.vector.memset(zt[:], 0.0)
        padw = []
        for s in range(2):
            d = nc.sync.dma_start(
                bb[s].ap()[:, DC:DCP].rearrange("(n p) c -> p n c", p=128),
                zt[:].rearrange("p (n c) -> p n c", n=NT),
            )
            if prev_tail is not None:
                add_dep_helper(d.ins, prev_tail.ins, sync=True, reason="pass order")
            padw.append(d)

        def umat(ps, I, k):
            """U_k tile I into psum [128, DC]: 7 disjoint-column matmuls."""
            for r in range(7):
                cols = 128 if r < 6 else 96
                nc.tensor.matmul(
                    ps[:, r * 128 : r * 128 + cols],
                    x0sb[r][:, I * 128 : (I + 1) * 128],
                    wksb[:, k * 128 : k * 128 + cols],
                    start=True,
                    stop=True,
                )

        # ---- mini-phase: B_7 = U_7 -> slab 1 ----
        writes = []
        with (
            tc.tile_pool(name=f"mp{_pass}", bufs=3) as mp,
            tc.tile_pool(name=f"mps{_pass}", bufs=2, space="PSUM") as mps,
        ):
            for I in range(NT):
                ps = mps.tile([128, DC], F32)
                umat(ps, I, 7)
                st = mp.tile([128, DC], BF16)
                nc.vector.tensor_copy(st[:], ps[:])
                d = nc.sync.dma_start(
                    bb[1].ap()[I * 128 : (I + 1) * 128, 0:DC], st[:]
                )
                if prev_tail is not None and I == 0:
                    add_dep_helper(
                        d.ins, prev_tail.ins, sync=True, reason="pass order"
                    )
                writes.append(d)
        fence = nc.gpsimd.engine_nop()
        for d in writes + padw:
            add_dep_helper(fence.ins, d.ins, sync=True, reason="lvl7 fence")
        fences = {7: fence}

        # ---- Clenshaw sweeps: k = 6..0 ----
        gp = ctx.enter_context(tc.tile_pool(name=f"g{_pass}", bufs=2))
        pp = ctx.enter_context(tc.tile_pool(name=f"ps{_pass}", bufs=2, space="PSUM"))
        pp2 = ctx.enter_context(tc.tile_pool(name=f"ps2{_pass}", bufs=2, space="PSUM"))
        bp = ctx.enter_context(tc.tile_pool(name=f"bp{_pass}", bufs=3))
        tp = ctx.enter_context(tc.tile_pool(name=f"tp{_pass}", bufs=2))
        sp = ctx.enter_context(tc.tile_pool(name=f"sp{_pass}", bufs=3))

        for k in range(6, -1, -1):
            src = bb[(k + 1) % 2]
            dst = bb[k % 2]
            writes = []
            for I in range(NT):
                t = tiles[I]
                jt, n = t["jt"], t["n"]
                if "gather" not in skip:
                    g = gp.tile([128, JTMAX, DCP], BF16, name="g")
                    if GATHER_MODE == "indirect":
                        for j in range(jt):
                            gd = nc.gpsimd.indirect_dma_start(
                                out=g[:, j, :],
                                out_offset=None,
                                in_=src.ap(),
                                in_offset=bass.IndirectOffsetOnAxis(
                                    ap=g32sb[:, t["gcol"] + j : t["gcol"] + j + 1],
                                    axis=0,
                                ),
                            )
                            add_dep_helper(
                                gd.ins, fences[k + 1].ins, sync=True,
                                reason="lvl raw",
                            )
                    else:
                        for c in range(0, jt, 2):
                            ng = min(2, jt - c)
                            gd = nc.gpsimd.dma_gather(
                                g[:, c : c + ng, :], src.ap(),
                                gisb[
                                    :,
                                    t["icol"] + c * 8 : t["icol"] + (c + ng) * 8,
                                ],
                                ng * 128, nreg[ng], DCP,
                                queue_num=(c // 2) % 4,
                            )
                            add_dep_helper(
                                gd.ins, fences[k + 1].ins, sync=True,
                                reason="lvl raw",
                            )
                    rhs_of = lambda j, off, nn: g[:, j, off : off + nn]
                else:
                    rhs_of = lambda j, off, nn: selsb[:, off : off + nn]
                ps2 = pp2.tile([128, DC], F32)
                if "umat" not in skip:
                    umat(ps2, I, k)
                ps = pp.tile([128, DC], F32)
                if "spmm" not in skip:
                    for j in range(jt):
                        lhsT = selsb[
                            :, t["scol"] + j * 128 : t["scol"] + (j + 1) * 128
                        ]
                        for off, nn in ((0, 512), (512, DC - 512)):
                            nc.tensor.matmul(
                                ps[:, off : off + nn],
                                lhsT,
                                rhs_of(j, off, nn),
                                start=(j == 0),
                                stop=(j == jt - 1),
                            )
                else:
                    nc.tensor.matmul(
                        ps[:, 0:512], selsb[:, 0:128], rhs_of(0, 0, 512),
                        start=True, stop=True,
                    )
                if "combine" in skip or ("io" in skip and k != 6):
                    st = sp.tile([128, DC], BF16)
                    nc.vector.tensor_copy(st[:], ps[:])
                elif k == 6:  # B_8 = 0; DVE can read only one PSUM operand
                    u6 = tp.tile([128, DC], F32, tag="u6")
                    nc.scalar.copy(u6[:], ps2[:])
                    st = sp.tile([128, DC], BF16)
                    nc.vector.scalar_tensor_tensor(
                        st[:], ps[:], 2.0, u6[:], MUL, ADD
                    )
                else:
                    bprev = bp.tile([128, DC], BF16)
                    # ACT queue: keeps the prefetch from queuing behind this
                    # sweep's B writes on the in-order SP queue
                    bd = nc.scalar.dma_start(
                        bprev[:], dst.ap()[I * 128 : (I + 1) * 128, 0:DC]
                    )
                    if k + 2 in fences:
                        add_dep_helper(
                            bd.ins, fences[k + 2].ins, sync=True, reason="bprev raw"
                        )
                    tt = tp.tile([128, DC], F32)
                    nc.vector.scalar_tensor_tensor(
                        tt[:], ps[:], 2.0 if k > 0 else 1.0, bprev[:], MUL, SUB
                    )
                    st = sp.tile([128, DC], BF16)
                    if k > 0:
                        nc.vector.tensor_add(st[:], tt[:], ps2[:])
                    else:
                        t2 = tp.tile([128, DC], F32, tag="t2")
                        nc.vector.tensor_add(t2[:], tt[:], ps2[:])
                        nc.vector.tensor_add(st[:], t2[:], brsb[:])
                if k > 0:
                    writes.append(
                        nc.sync.dma_start(
                            dst.ap()[I * 128 : (I + 1) * 128, 0:DC], st[:]
                        )
                    )
                else:
                    writes.append(
                        nc.sync.dma_start(
                            sout.ap()[I * 128 : (I + 1) * 128, :], st[:]
                        )
                    )
            fence = nc.gpsimd.engine_nop()
            for d in writes:
                add_dep_helper(fence.ins, d.ins, sync=True, reason=f"lvl{k} fence")
            fences[k] = fence
        prev_tail = fences[0]
        ctx.pop_all().close()

    _fix_excess_waits(nc)
    lower_extended_insts(nc)
    return nc


_REPLICATED = {"wk", "seld", "gidxd", "gidx32d", "brep"}


def _make_runner(nc):
    """Like bass2jax.run_bass_via_pjrt, but the jitted sharded callable is
    built once and reused, and core-invariant inputs are replicated via
    P() specs instead of concatenated 8x."""
    import jax
    from jax.experimental.shard_map import shard_map
    from jax.sharding import Mesh, PartitionSpec

    bass2jax.install_neuronx_cc_hook()
    partition_name = (
        nc.partition_id_tensor.name if nc.partition_id_tensor else None
    )
    in_names, out_names, out_avals, zero_outs = [], [], [], []
    for alloc in nc.m.functions[0].allocations:
        if not isinstance(alloc, mybir.MemoryLocationSet):
            continue
        name = alloc.memorylocations[0].name
        if alloc.kind == "ExternalInput":
            if name != partition_name:
                in_names.append(name)
        elif alloc.kind == "ExternalOutput":
            shape = tuple(alloc.tensor_shape)
            dtype = mybir.dt.np(alloc.dtype)
            out_names.append(name)
            out_avals.append(jax.core.ShapedArray(shape, dtype))
            zero_outs.append(np.zeros(shape, dtype))
    n_params = len(in_names)
    n_outs = len(out_avals)
    all_names = in_names + out_names + ([partition_name] if partition_name else [])
    donate = tuple(range(n_params, n_params + n_outs))

    def _body(*args):
        operands = list(args)
        if partition_name is not None:
            operands.append(bass2jax.partition_id_tensor())
        return tuple(
            bass2jax._bass_exec_p.bind(
                *operands,
                out_avals=tuple(out_avals),
                in_names=tuple(all_names),
                out_names=tuple(out_names),
                lowering_input_output_aliases=(),
                sim_require_finite=True,
                sim_require_nnan=True,
                nc=nc,
            )
        )

    devices = jax.devices()[:NCORES]
    mesh = Mesh(np.asarray(devices), ("core",))
    in_specs = tuple(
        PartitionSpec() if nm in _REPLICATED else PartitionSpec("core")
        for nm in in_names
    ) + (PartitionSpec("core"),) * n_outs
    out_specs = (PartitionSpec("core"),) * n_outs
    sharded = jax.jit(
        shard_map(
            _body, mesh=mesh, in_specs=in_specs, out_specs=out_specs,
            check_rep=False,
        ),
        keep_unused=True,
    )
    from jax.sharding import NamedSharding

    shardings = {
        nm: NamedSharding(
            mesh, PartitionSpec() if nm in _REPLICATED else PartitionSpec("core")
        )
        for nm in in_names
    }
    core_sharding = NamedSharding(mesh, PartitionSpec("core"))
    zero_bufs = [
        jax.device_put(
            np.zeros((NCORES * z.shape[0], *z.shape[1:]), z.dtype), core_sharding
        )
        for z in zero_outs
    ]
    dev_cache = {}

    def _staged(nm, host_arr):
        ent = dev_cache.get(nm)
        if ent is not None and ent[0] is host_arr:
            return ent[1]
        darr = jax.device_put(host_arr, shardings[nm])
        dev_cache[nm] = (host_arr, darr)
        return darr

    concat_cache = {}

    def run(in_maps):
        args = []
        for nm in in_names:
            if nm in _REPLICATED:
                args.append(_staged(nm, in_maps[0][nm]))
            else:
                ck = concat_cache.get(nm)
                if ck is None or ck[0] is not in_maps[0][nm]:
                    cat = np.concatenate([m[nm] for m in in_maps], axis=0)
                    concat_cache[nm] = ck = (in_maps[0][nm], cat)
                args.append(_staged(nm, ck[1]))
        args.extend(zero_bufs)
        run.last_args = args
        out_arrs = sharded(*args)
        return [
            {
                nm: np.asarray(out_arrs[i]).reshape(
                    NCORES, *out_avals[i].shape
                )[c]
                for i, nm in enumerate(out_names)
            }
            for c in range(NCORES)
        ]

    run.last_args = None
    run.sharded = sharded
    return run


_NPHI = 5  # high-pass count for the timing differencing


def hw_exec_time_ns(trials=16):
    """Measured device execution time for one full kernel pass: compile the
    same program with the whole algorithm run once vs _NPHI times
    back-to-back in one NEFF, time both with device-resident args, and
    difference to cancel dispatch overhead (the RPC floor through axon is
    ~75ms, so the extra passes amplify the device-time signal well above
    the jitter)."""
    import time

    run = _CACHE.get("run")
    if run is None or run.last_args is None:
        raise RuntimeError("call kernel() first")
    if "runN" not in _CACHE:
        _CACHE["runN"] = _make_runner(_build_nc(_CACHE["tiles"], npasses=_NPHI))
    runN = _CACHE["runN"]

    def timed(fn):
        best = float("inf")
        for _ in range(trials):
            t0 = time.time()
            outs = fn(*run.last_args)
            for o in outs:
                o.block_until_ready()
            best = min(best, time.time() - t0)
        return best

    timed(run.sharded); timed(runN.sharded)  # warm both executables
    timed(run.sharded); timed(runN.sharded)  # twice: absorb cold-load outliers
    t1 = timed(run.sharded)
    tN = timed(runN.sharded)
    return (tN - t1) * 1e9 / (_NPHI - 1)


def _host_prep(inputs, weight, bias, lap_vals, lap_rows, lap_cols):
    bf = ml_dtypes.bfloat16
    tiles, sel, gidx, gidx32 = _build_schedule(lap_rows, lap_cols, lap_vals)

    # per-level block-diagonal weights: wk[(g,fin), k*128 + (g,fo)] = W_k[fin,fo]
    Wf = np.asarray(weight, dtype=np.float32)  # [K, FIN, FOUT]
    wk = np.zeros((128, KK * 128), np.float32)
    for k in range(KK):
        for g in range(4):
            wk[g * 32 : (g + 1) * 32, k * 128 + g * 32 : k * 128 + (g + 1) * 32] = (
                Wf[k]
            )
    wk = wk.astype(bf)

    brep = np.broadcast_to(
        np.tile(np.asarray(bias, dtype=np.float32), DPC)[None, :], (128, DC)
    ).copy()

    # per-core x0^T shards: x0t_m[d'*32+fin, v] = inputs[0, fin, v, 27m+d']
    x = np.asarray(inputs, dtype=np.float32).reshape(FIN, V, DP)
    in_maps = []
    for m in range(NCORES):
        xm = x[:, :, DPC * m : DPC * (m + 1)]  # [fin, v, d']
        x0t_m = np.ascontiguousarray(
            xm.transpose(2, 0, 1).reshape(DC, V).astype(bf)
        )
        in_maps.append(
            {
                "x0t": x0t_m, "wk": wk, "seld": sel, "gidxd": gidx,
                "gidx32d": gidx32, "brep": brep,
            }
        )
    return tiles, in_maps


def kernel(inputs, weight, bias, lap_vals, lap_rows, lap_cols):
    import hashlib

    args = [
        np.asarray(a)
        for a in (inputs, weight, bias, lap_vals, lap_rows, lap_cols)
    ]
    key = b"".join(
        hashlib.blake2b(np.ascontiguousarray(a).view(np.uint8), digest_size=16).digest()
        for a in args
    )
    if _CACHE.get("prep_key") != key:
        tiles, in_maps = _host_prep(*args)
        # program structure depends only on the laplacian index pattern
        skey = tuple((t["jt"], t["n"]) for t in tiles)
        if _CACHE.get("skey") != skey:
            _CACHE["run"] = _make_runner(_build_nc(tiles))
            _CACHE.pop("run2", None)
            _CACHE["skey"] = skey
        _CACHE["tiles"] = tiles
        _CACHE["in_maps"] = in_maps
        _CACHE["prep_key"] = key
    results = _CACHE["run"](_CACHE["in_maps"])
    out = np.zeros((FOUT, V, DP), dtype=np.float32)
    for m in range(NCORES):
        S = results[m]["sout"].astype(np.float32).reshape(V, DPC, FOUT)
        out[:, :, DPC * m : DPC * (m + 1)] = S.transpose(2, 0, 1)
    return out.reshape(1, FOUT, V, 6, 6, 6)
